# revision 13
# baseline (speedup 1.0000x reference)
"""Trainium2 Bass kernel for single-head attention with input projections.

    query = q @ Wq + bq ; key = k @ Wk + bk ; value = v @ Wv + bv
    out   = softmax(query @ key.T / sqrt(H)) @ value
    (q, k, v: [4096, 1024] fp32; Wq/Wk/Wv: [1024, 1024]; out: [4096, 1024])

Runs on 8 NeuronCores (SPMD via run_bass_kernel_spmd).  HW exec time
~203 us/core (measured via NTFF profiles), output max-rel error
~6.3e-3 (fro ~5.0e-3) vs the fp32 reference.

Design — both projection chains are re-associated so the two big
[4096, 1024] operands enter the PE raw, with no transposes of k or v,
no key/value projection passes over the long sequence, and no
collectives:

  - scores^T = k^T-tiles @ t, with u = q@Wq + bq and t = u@Wk^T
    computed per core on its 512 q-rows (128 small matmuls).  bk only
    shifts scores rows uniformly and provably cancels in softmax; the
    1/sqrt(H) scale rides the Exp activation.
  - out = (softmax_w @ v) @ Wv: cv = w @ v consumes raw v in natural
    [sk, d] layout; cv ([512, 1024] per core) is transposed on-chip
    with 32 PE transpose ops and projected by Wv at the end, where
    1/rowsum (and bv) are applied.
  - Row sums come from one accumulating ones^T @ exp_w matmul per
    score tile (sums replicated across partitions, recovered
    per-partition with 4 PE transposes).

Sharding: q-rows split across the 8 cores (each core computes its 512
output rows independently); k^T and v are replicated by the host as
bf16 streams (pure layout/dtype transforms, zero host FLOPs).

Precision: the streamed operands (k^T tiles, exp weights, raw v) and
the u/t chain are bf16; cv and the final Wv projection run in float32r
(fp32 storage at bf16 rate, TF32-like precision); all accumulation is
fp32 PSUM.  Softmax skips max-subtraction — exact softmax math
otherwise.  Range contract: valid while scaled scores stay below ~85
(exp must not overflow fp32).  For the reference distribution
(unit-normal q/k/v, Wq/Wk scaled 1/sqrt(D)) scores/sqrt(H) ~ N(0,1)
with max ~5.5, a 15x margin; inputs scaled beyond ~4x sigma would
need a max-subtraction pass instead.

Schedule notes: uT accumulates m-major across all 8 PSUM banks so its
groups complete as the DMA lands; k^T/v stream as double-buffered
512-column chunks; per-phase PSUM pools are sized to at most 8 banks;
the Wv load is deferred out of the congested head window.
"""
import numpy as np

import concourse.bacc as bacc
import concourse.mybir as mybir
import concourse.tile as tile
from concourse.bass_utils import run_bass_kernel_spmd

F32 = mybir.dt.float32
F32R = mybir.dt.float32r
BF16 = mybir.dt.bfloat16
AF = mybir.ActivationFunctionType

S = 4096
D = 1024
H = 1024
NCORES = 8
SQ = S // NCORES
CH = 512
NCH = S // CH
NT = D // 128
NJ = H // 128
NB = SQ // 128
INV_SQRT_H = 1.0 / np.sqrt(np.float32(H))


def build_program(apply_bq: bool, apply_bv: bool):
    nc = bacc.Bacc("TRN2", target_bir_lowering=False, debug=False,
                   enable_asserts=False, num_devices=NCORES)

    qt = nc.dram_tensor("qt", [D, SQ], BF16, kind="ExternalInput").ap()
    ktf = nc.dram_tensor("ktf", [D, S], BF16, kind="ExternalInput").ap()
    vf = nc.dram_tensor("vf", [S, D], BF16, kind="ExternalInput").ap()
    wq = nc.dram_tensor("wq", [D, H], BF16, kind="ExternalInput").ap()
    wkt = nc.dram_tensor("wkt", [H, D], BF16, kind="ExternalInput").ap()
    wv = nc.dram_tensor("wv", [D, H], F32R, kind="ExternalInput").ap()
    bq_r = nc.dram_tensor("bq_r", [NJ, 128], F32, kind="ExternalInput").ap()
    bv_d = nc.dram_tensor("bv_d", [1, H], F32, kind="ExternalInput").ap()
    ones_d = nc.dram_tensor("ones_d", [128, 4], BF16, kind="ExternalInput").ap()
    ident_d = nc.dram_tensor("ident_d", [128, 128], F32R, kind="ExternalInput").ap()
    ones_f = nc.dram_tensor("ones_f", [1, 128], F32, kind="ExternalInput").ap()
    ones_bb = nc.dram_tensor("ones_bb", [128, 128], BF16, kind="ExternalInput").ap()
    out = nc.dram_tensor("out", [SQ, H], F32, kind="ExternalOutput").ap()

    with tile.TileContext(nc) as tc:
        with tc.tile_pool(name="persist", bufs=1) as pp:
            tT = pp.tile([128, NT, SQ], BF16)       # ((q@Wq + bq) @ Wk^T)^T
            ones_sb = pp.tile([128, 4], BF16)
            nc.sync.dma_start(ones_sb[:], ones_d[:])
            ident_sb = pp.tile([128, 128], F32R)
            nc.sync.dma_start(ident_sb[:], ident_d[:])
            ones_bb_sb = pp.tile([128, 128], BF16)
            nc.sync.dma_start(ones_bb_sb[:], ones_bb[:])
            inv_sb = pp.tile([128, NB], F32)
            sums_sb = pp.tile([128, NB], F32)
            wv_sb = pp.tile([128, NT, H], F32R)     # used in the last phase
            if apply_bq:
                bq_sb = pp.tile([128, NJ], F32)
                nc.sync.dma_start(bq_sb[:], bq_r.rearrange("t p -> p t"))
            if apply_bv:
                # bv is applied post-normalize; broadcast it across
                # partitions once via a K=1 ones matmul.
                bv_row = pp.tile([1, H], F32)
                nc.sync.dma_start(bv_row[:], bv_d[:])
                onef = pp.tile([1, 128], F32)
                nc.sync.dma_start(onef[:], ones_f[:])
                bv_bcast = pp.tile([128, H], F32)
                with tc.tile_pool(name="bv_ps", bufs=2, space="PSUM") as bv_ps:
                    for half in range(2):
                        hs = slice(512 * half, 512 * (half + 1))
                        psb = bv_ps.tile([128, 512], F32)
                        nc.tensor.matmul(psb[:], onef[:], bv_row[0:1, hs],
                                         start=True, stop=True)
                        nc.scalar.activation(bv_bcast[:, hs], psb[:], AF.Copy)

            # ---- P0: uT then tT (128 matmuls, gated by 10 MB of DMA) ----
            with (
                tc.tile_pool(name="p0", bufs=1) as p0,
                tc.tile_pool(name="p0_ps", bufs=2, space="PSUM") as p0_ps,
            ):
                wq_sb = p0.tile([128, NT, H], BF16)
                qt_sb = p0.tile([128, NT, SQ], BF16)
                for t in range(NT):
                    ts_ = slice(128 * t, 128 * (t + 1))
                    nc.sync.dma_start(wq_sb[:, t, :], wq[ts_, :])
                    nc.sync.dma_start(qt_sb[:, t, :], qt[ts_, :])
                wkt_sb = p0.tile([128, NJ, D], BF16)
                for m in range(NJ):
                    nc.sync.dma_start(wkt_sb[:, m, :], wkt[128 * m:128 * (m + 1), :])
                uT = p0.tile([128, NJ, SQ], BF16)
                ups = [p0_ps.tile([128, SQ], F32, name=f"ups{j}", tag="ups", bufs=8)
                       for j in range(NJ)]
                for t in range(NT):
                    for j in range(NJ):
                        nc.tensor.matmul(ups[j][:], wq_sb[:, t, 128 * j:128 * (j + 1)],
                                         qt_sb[:, t, :], start=(t == 0), stop=(t == NT - 1))
                for j in range(NJ):
                    if apply_bq:
                        nc.scalar.activation(uT[:, j, :], ups[j][:], AF.Identity,
                                             bias=bq_sb[:, j:j + 1])
                    else:
                        nc.scalar.activation(uT[:, j, :], ups[j][:], AF.Copy)

                for j in range(NT):
                    ps = p0_ps.tile([128, SQ], F32, tag="ups", bufs=8)
                    for m in range(NJ):
                        nc.tensor.matmul(ps[:], wkt_sb[:, m, 128 * j:128 * (j + 1)],
                                         uT[:, m, :], start=(m == 0), stop=(m == NJ - 1))
                    nc.scalar.activation(tT[:, j, :], ps[:], AF.Copy)

            # ---- C0 + C1 ----
            with tc.tile_pool(name="pc_all", bufs=1) as pc_all:
              expT = pc_all.tile([128, S // 128, SQ], BF16)
              # C0: scoresT from raw kT chunks -> exp -> row sums
              with (
                tc.tile_pool(name="pb_dbl", bufs=4) as pb_dbl,
                tc.tile_pool(name="pb_ps", bufs=2, space="PSUM") as pb_ps,
                tc.tile_pool(name="psum_sum", bufs=1, space="PSUM") as psum_sum,
              ):
                sums32_ps = psum_sum.tile([128, 512], F32, name="sums32")
                for c in range(NCH):
                    kt_ch = pb_dbl.tile([128, NT, CH], BF16, tag="kt")
                    for t in range(NT):
                        nc.sync.dma_start(
                            kt_ch[:, t, :],
                            ktf[128 * t:128 * (t + 1), CH * c:CH * (c + 1)])
                    for u in range(CH // 128):
                        idx = (CH // 128) * c + u
                        ps = pb_ps.tile([128, SQ], F32, tag="sps", bufs=4)
                        for t in range(NT):
                            nc.tensor.matmul(ps[:], kt_ch[:, t, 128 * u:128 * (u + 1)],
                                             tT[:, t, :], start=(t == 0), stop=(t == NT - 1))
                        nc.scalar.activation(expT[:, idx, :], ps[:], AF.Exp,
                                             scale=float(INV_SQRT_H))
                        nc.tensor.matmul(sums32_ps[:], ones_bb_sb[:],
                                         expT[:, idx, :],
                                         start=(idx == 0), stop=(idx == S // 128 - 1))
                # every row of sums32 is the full rowsum vector; transpose
                # 128-column blocks to get per-partition sums per sq-block.
                sums_bc = pc_all.tile([128, 512], F32R, name="sums_bc")
                nc.scalar.activation(sums_bc[:], sums32_ps[:], AF.Copy)
                with tc.tile_pool(name="sum_tp", bufs=2, space="PSUM") as sum_tp:
                    for b in range(NB):
                        tpb = sum_tp.tile([128, 128], F32R, tag="stp")
                        nc.tensor.transpose(tpb[:], sums_bc[:, 128 * b:128 * (b + 1)],
                                            ident_sb[:])
                        nc.vector.tensor_copy(sums_sb[:, b:b + 1], tpb[:, 0:1])
              nc.vector.reciprocal(inv_sb[:], sums_sb[:])

              # C1a: cv = exp_w @ v over raw v chunks
              cv_sb = [pc_all.tile([128, D], F32R, name=f"cv{b}") for b in range(NB)]
              with (
                tc.tile_pool(name="pv_dbl", bufs=3) as pv_dbl,
                tc.tile_pool(name="pcv", bufs=1, space="PSUM") as pcv,
              ):
                cv_ps = [pcv.tile([128, 512], F32, name=f"cvp{b}_{dh}", tag=f"cvp{b}_{dh}")
                         for b in range(NB) for dh in range(2)]
                for c in range(NCH):
                    v_ch = pv_dbl.tile([128, CH // 128, D], BF16, tag="v")
                    nc.sync.dma_start(
                        v_ch[:],
                        vf[CH * c:CH * (c + 1), :].rearrange("(u p) d -> p u d", p=128))
                    for u in range(CH // 128):
                        idx = (CH // 128) * c + u
                        for dh in range(2):
                            for b in range(NB):
                                nc.tensor.matmul(
                                    cv_ps[2 * b + dh][:],
                                    expT[:, idx, 128 * b:128 * (b + 1)],
                                    v_ch[:, u, 512 * dh:512 * (dh + 1)],
                                    start=(idx == 0), stop=(idx == S // 128 - 1))
                for b in range(NB):
                    for dh in range(2):
                        nc.scalar.activation(cv_sb[b][:, 512 * dh:512 * (dh + 1)],
                                             cv_ps[2 * b + dh][:], AF.Copy)

              # C1b+c: transpose cv, project with Wv, normalize, store
              with (
                tc.tile_pool(name="pf", bufs=1) as pf,
                tc.tile_pool(name="pf_out", bufs=2) as pf_out,
                tc.tile_pool(name="pf_tp", bufs=4, space="PSUM") as pf_tp,
                tc.tile_pool(name="pf_ctx", bufs=2, space="PSUM") as pf_ctx,
              ):
                cvT = pf.tile([128, NT, SQ], F32R)
                for t in range(NT):
                    nc.sync.dma_start(wv_sb[:, t, :], wv[128 * t:128 * (t + 1), :])
                for b in range(NB):
                    for t in range(NT):
                        tp = pf_tp.tile([128, 128], F32R, tag="tp")
                        nc.tensor.transpose(tp[:], cv_sb[b][:, 128 * t:128 * (t + 1)],
                                            ident_sb[:])
                        nc.vector.tensor_copy(cvT[:, t, 128 * b:128 * (b + 1)], tp[:])
                    for h_ in range(2):
                        hs = slice(512 * h_, 512 * (h_ + 1))
                        ps = pf_ctx.tile([128, 512], F32, tag="ctx")
                        for t in range(NT):
                            nc.tensor.matmul(ps[:], cvT[:, t, 128 * b:128 * (b + 1)],
                                             wv_sb[:, t, hs], start=(t == 0), stop=(t == NT - 1))
                        out_t = pf_out.tile([128, 512], F32, tag="out")
                        nc.vector.tensor_scalar_mul(out_t[:], ps[:], inv_sb[:, b:b + 1])
                        if apply_bv:
                            nc.vector.tensor_tensor(out_t[:], out_t[:], bv_bcast[:, hs],
                                                    op=mybir.AluOpType.add)
                        nc.sync.dma_start(out[128 * b:128 * (b + 1), hs], out_t[:])

    nc.compile()
    return nc


_CACHE = {}


def _get_program(apply_bq: bool, apply_bv: bool):
    key = (apply_bq, apply_bv)
    if key not in _CACHE:
        _CACHE[key] = build_program(apply_bq, apply_bv)
    return _CACHE[key]


def _prepare_in_maps(ins: dict) -> list:
    import ml_dtypes
    q = np.asarray(ins["q"], np.float32)
    k = np.asarray(ins["k"], np.float32)
    v = np.asarray(ins["v"], np.float32)
    assert q.shape == (S, D) and k.shape == (S, D) and v.shape == (S, D)

    qT = np.ascontiguousarray(q.T).astype(ml_dtypes.bfloat16)
    kT_bf = np.ascontiguousarray(k.T).astype(ml_dtypes.bfloat16)
    v_bf = v.astype(ml_dtypes.bfloat16)
    Wq = np.ascontiguousarray(np.asarray(ins["Wq"], np.float32)).astype(ml_dtypes.bfloat16)
    WkT = np.ascontiguousarray(np.asarray(ins["Wk"], np.float32).T).astype(ml_dtypes.bfloat16)
    Wv = np.ascontiguousarray(np.asarray(ins["Wv"], np.float32))
    bq = np.asarray(ins["bq"], np.float32).reshape(H)
    bv = np.asarray(ins["bv"], np.float32).reshape(H)

    bq_r = np.ascontiguousarray(bq.reshape(NJ, 128))
    bv_d = np.ascontiguousarray(bv.reshape(1, H))
    ones_np = np.ones((128, 4), ml_dtypes.bfloat16)
    ident_np = np.eye(128, dtype=np.float32)

    in_maps = []
    for i in range(NCORES):
        sl = slice(SQ * i, SQ * (i + 1))
        in_maps.append({
            "qt": np.ascontiguousarray(qT[:, sl]),
            "ktf": kT_bf, "vf": v_bf,
            "wq": Wq, "wkt": WkT, "wv": Wv,
            "bq_r": bq_r, "bv_d": bv_d, "ones_d": ones_np, "ident_d": ident_np,
            "ones_f": np.ones((1, 128), np.float32),
            "ones_bb": np.ones((128, 128), ml_dtypes.bfloat16),
        })
    return in_maps


def kernel(q, k, v, Wq, bq, Wk, bk, Wv, bv) -> np.ndarray:
    # bk contributes only per-row constants to scores and cancels in softmax.
    ins = {"q": q, "k": k, "v": v, "Wq": Wq, "bq": bq, "Wk": Wk,
           "Wv": Wv, "bv": bv}
    apply_bq = bool(np.any(np.asarray(bq)))
    apply_bv = bool(np.any(np.asarray(bv)))
    nc = _get_program(apply_bq, apply_bv)
    in_maps = _prepare_in_maps(ins)
    res = run_bass_kernel_spmd(nc, in_maps, core_ids=list(range(NCORES)))
    return np.concatenate([res.results[i]["out"] for i in range(NCORES)], axis=0)


# revision 14
# speedup vs baseline: 1.0077x; 1.0077x over previous
"""Trainium2 Bass kernel for single-head attention with input projections.

    query = q @ Wq + bq ; key = k @ Wk + bk ; value = v @ Wv + bv
    out   = softmax(query @ key.T / sqrt(H)) @ value
    (q, k, v: [4096, 1024] fp32; Wq/Wk/Wv: [1024, 1024]; out: [4096, 1024])

Runs on 8 NeuronCores (SPMD via run_bass_kernel_spmd).  HW exec time
~203 us/core (measured via NTFF profiles), output max-rel error
~6.3e-3 (fro ~5.0e-3) vs the fp32 reference.

Design — both projection chains are re-associated so the two big
[4096, 1024] operands enter the PE raw, with no transposes of k or v,
no key/value projection passes over the long sequence, and no
collectives:

  - scores^T = k^T-tiles @ t, with u = q@Wq + bq and t = u@Wk^T
    computed per core on its 512 q-rows (128 small matmuls).  bk only
    shifts scores rows uniformly and provably cancels in softmax; the
    1/sqrt(H) scale rides the Exp activation.
  - out = (softmax_w @ v) @ Wv: cv = w @ v consumes raw v in natural
    [sk, d] layout; cv ([512, 1024] per core) is transposed on-chip
    with 32 PE transpose ops and projected by Wv at the end, where
    1/rowsum (and bv) are applied.
  - Row sums come from one accumulating ones^T @ exp_w matmul per
    score tile (sums replicated across partitions, recovered
    per-partition with 4 PE transposes).

Sharding: q-rows split across the 8 cores (each core computes its 512
output rows independently); k^T and v are replicated by the host as
bf16 streams (pure layout/dtype transforms, zero host FLOPs).

Precision: the streamed operands (k^T tiles, exp weights, raw v) and
the u/t chain are bf16; cv and the final Wv projection run in float32r
(fp32 storage at bf16 rate, TF32-like precision); all accumulation is
fp32 PSUM.  Softmax skips max-subtraction — exact softmax math
otherwise.  Range contract: valid while scaled scores stay below ~85
(exp must not overflow fp32).  For the reference distribution
(unit-normal q/k/v, Wq/Wk scaled 1/sqrt(D)) scores/sqrt(H) ~ N(0,1)
with max ~5.5, a 15x margin; inputs scaled beyond ~4x sigma would
need a max-subtraction pass instead.

Schedule notes: uT accumulates m-major across all 8 PSUM banks so its
groups complete as the DMA lands; k^T/v stream as double-buffered
512-column chunks; per-phase PSUM pools are sized to at most 8 banks;
the Wv load is deferred out of the congested head window.
"""
import numpy as np

import concourse.bacc as bacc
import concourse.mybir as mybir
import concourse.tile as tile
from concourse.bass_utils import run_bass_kernel_spmd

F32 = mybir.dt.float32
F32R = mybir.dt.float32r
BF16 = mybir.dt.bfloat16
AF = mybir.ActivationFunctionType

S = 4096
D = 1024
H = 1024
NCORES = 8
SQ = S // NCORES
CH = 512
NCH = S // CH
NT = D // 128
NJ = H // 128
NB = SQ // 128
INV_SQRT_H = 1.0 / np.sqrt(np.float32(H))


def build_program(apply_bq: bool, apply_bv: bool):
    nc = bacc.Bacc("TRN2", target_bir_lowering=False, debug=False,
                   enable_asserts=False, num_devices=NCORES)

    qt = nc.dram_tensor("qt", [D, SQ], BF16, kind="ExternalInput").ap()
    ktf = nc.dram_tensor("ktf", [D, S], BF16, kind="ExternalInput").ap()
    vf = nc.dram_tensor("vf", [S, D], BF16, kind="ExternalInput").ap()
    wq = nc.dram_tensor("wq", [D, H], BF16, kind="ExternalInput").ap()
    wkt = nc.dram_tensor("wkt", [H, D], BF16, kind="ExternalInput").ap()
    wv = nc.dram_tensor("wv", [D, H], F32R, kind="ExternalInput").ap()
    bq_r = nc.dram_tensor("bq_r", [NJ, 128], F32, kind="ExternalInput").ap()
    bv_d = nc.dram_tensor("bv_d", [1, H], F32, kind="ExternalInput").ap()
    ident_d = nc.dram_tensor("ident_d", [128, 128], F32R, kind="ExternalInput").ap()
    ones_f = nc.dram_tensor("ones_f", [1, 128], F32, kind="ExternalInput").ap()
    ones_bb = nc.dram_tensor("ones_bb", [128, 128], BF16, kind="ExternalInput").ap()
    out = nc.dram_tensor("out", [SQ, H], F32, kind="ExternalOutput").ap()

    with tile.TileContext(nc) as tc:
        with tc.tile_pool(name="persist", bufs=1) as pp:
            tT = pp.tile([128, NT, SQ], BF16)       # ((q@Wq + bq) @ Wk^T)^T
            ident_sb = pp.tile([128, 128], F32R)
            nc.sync.dma_start(ident_sb[:], ident_d[:])
            ones_bb_sb = pp.tile([128, 128], BF16)
            nc.sync.dma_start(ones_bb_sb[:], ones_bb[:])
            inv_sb = pp.tile([128, NB], F32)
            sums_sb = pp.tile([128, NB], F32)
            wv_sb = pp.tile([128, NT, H], F32R)     # used in the last phase
            if apply_bq:
                bq_sb = pp.tile([128, NJ], F32)
                nc.sync.dma_start(bq_sb[:], bq_r.rearrange("t p -> p t"))
            if apply_bv:
                # bv is applied post-normalize; broadcast it across
                # partitions once via a K=1 ones matmul.
                bv_row = pp.tile([1, H], F32)
                nc.sync.dma_start(bv_row[:], bv_d[:])
                onef = pp.tile([1, 128], F32)
                nc.sync.dma_start(onef[:], ones_f[:])
                bv_bcast = pp.tile([128, H], F32)
                with tc.tile_pool(name="bv_ps", bufs=2, space="PSUM") as bv_ps:
                    for half in range(2):
                        hs = slice(512 * half, 512 * (half + 1))
                        psb = bv_ps.tile([128, 512], F32)
                        nc.tensor.matmul(psb[:], onef[:], bv_row[0:1, hs],
                                         start=True, stop=True)
                        nc.scalar.activation(bv_bcast[:, hs], psb[:], AF.Copy)

            # ---- P0: uT then tT (128 matmuls, gated by 10 MB of DMA) ----
            with (
                tc.tile_pool(name="p0", bufs=1) as p0,
                tc.tile_pool(name="p0_ps", bufs=2, space="PSUM") as p0_ps,
            ):
                wq_sb = p0.tile([128, NT, H], BF16)
                qt_sb = p0.tile([128, NT, SQ], BF16)
                for t in range(NT):
                    ts_ = slice(128 * t, 128 * (t + 1))
                    nc.sync.dma_start(wq_sb[:, t, :], wq[ts_, :])
                    nc.sync.dma_start(qt_sb[:, t, :], qt[ts_, :])
                wkt_sb = p0.tile([128, NJ, D], BF16)
                for m in range(NJ):
                    nc.sync.dma_start(wkt_sb[:, m, :], wkt[128 * m:128 * (m + 1), :])
                uT = p0.tile([128, NJ, SQ], BF16)
                ups = [p0_ps.tile([128, SQ], F32, name=f"ups{j}", tag="ups", bufs=8)
                       for j in range(NJ)]
                for t in range(NT):
                    for j in range(NJ):
                        nc.tensor.matmul(ups[j][:], wq_sb[:, t, 128 * j:128 * (j + 1)],
                                         qt_sb[:, t, :], start=(t == 0), stop=(t == NT - 1))
                for j in range(NJ):
                    if apply_bq:
                        nc.scalar.activation(uT[:, j, :], ups[j][:], AF.Identity,
                                             bias=bq_sb[:, j:j + 1])
                    else:
                        nc.scalar.activation(uT[:, j, :], ups[j][:], AF.Copy)

                for j in range(NT):
                    ps = p0_ps.tile([128, SQ], F32, tag="ups", bufs=8)
                    for m in range(NJ):
                        nc.tensor.matmul(ps[:], wkt_sb[:, m, 128 * j:128 * (j + 1)],
                                         uT[:, m, :], start=(m == 0), stop=(m == NJ - 1))
                    nc.scalar.activation(tT[:, j, :], ps[:], AF.Copy)

            # ---- C0 + C1 ----
            with tc.tile_pool(name="pc_all", bufs=1) as pc_all:
              expT = pc_all.tile([128, S // 128, SQ], BF16)
              # C0: scoresT from raw kT chunks -> exp -> row sums
              with (
                tc.tile_pool(name="pb_dbl", bufs=4) as pb_dbl,
                tc.tile_pool(name="pb_ps", bufs=2, space="PSUM") as pb_ps,
                tc.tile_pool(name="psum_sum", bufs=1, space="PSUM") as psum_sum,
              ):
                sums32_ps = psum_sum.tile([128, 512], F32, name="sums32")
                for c in range(NCH):
                    kt_ch = pb_dbl.tile([128, NT, CH], BF16, tag="kt")
                    for t in range(NT):
                        nc.sync.dma_start(
                            kt_ch[:, t, :],
                            ktf[128 * t:128 * (t + 1), CH * c:CH * (c + 1)])
                    for u in range(CH // 128):
                        idx = (CH // 128) * c + u
                        ps = pb_ps.tile([128, SQ], F32, tag="sps", bufs=4)
                        for t in range(NT):
                            nc.tensor.matmul(ps[:], kt_ch[:, t, 128 * u:128 * (u + 1)],
                                             tT[:, t, :], start=(t == 0), stop=(t == NT - 1))
                        nc.scalar.activation(expT[:, idx, :], ps[:], AF.Exp,
                                             scale=float(INV_SQRT_H))
                        nc.tensor.matmul(sums32_ps[:], ones_bb_sb[:],
                                         expT[:, idx, :],
                                         start=(idx == 0), stop=(idx == S // 128 - 1))
                # every row of sums32 is the full rowsum vector; transpose
                # 128-column blocks to get per-partition sums per sq-block.
                sums_bc = pc_all.tile([128, 512], F32R, name="sums_bc")
                nc.scalar.activation(sums_bc[:], sums32_ps[:], AF.Copy)
                with tc.tile_pool(name="sum_tp", bufs=2, space="PSUM") as sum_tp:
                    for b in range(NB):
                        tpb = sum_tp.tile([128, 128], F32R, tag="stp")
                        nc.tensor.transpose(tpb[:], sums_bc[:, 128 * b:128 * (b + 1)],
                                            ident_sb[:])
                        nc.vector.tensor_copy(sums_sb[:, b:b + 1], tpb[:, 0:1])
              nc.vector.reciprocal(inv_sb[:], sums_sb[:])

              # C1a: cv = exp_w @ v over raw v chunks
              cv_sb = [pc_all.tile([128, D], F32R, name=f"cv{b}") for b in range(NB)]
              with (
                tc.tile_pool(name="pv_dbl", bufs=3) as pv_dbl,
                tc.tile_pool(name="pcv", bufs=1, space="PSUM") as pcv,
              ):
                cv_ps = [pcv.tile([128, 512], F32, name=f"cvp{b}_{dh}", tag=f"cvp{b}_{dh}")
                         for b in range(NB) for dh in range(2)]
                for c in range(NCH):
                    v_ch = pv_dbl.tile([128, CH // 128, D], BF16, tag="v")
                    nc.sync.dma_start(
                        v_ch[:],
                        vf[CH * c:CH * (c + 1), :].rearrange("(u p) d -> p u d", p=128))
                    for u in range(CH // 128):
                        idx = (CH // 128) * c + u
                        for dh in range(2):
                            for b in range(NB):
                                nc.tensor.matmul(
                                    cv_ps[2 * b + dh][:],
                                    expT[:, idx, 128 * b:128 * (b + 1)],
                                    v_ch[:, u, 512 * dh:512 * (dh + 1)],
                                    start=(idx == 0), stop=(idx == S // 128 - 1))
                for b in range(NB):
                    for dh in range(2):
                        nc.scalar.activation(cv_sb[b][:, 512 * dh:512 * (dh + 1)],
                                             cv_ps[2 * b + dh][:], AF.Copy)

              # C1b+c: transpose cv, project with Wv, normalize, store
              with (
                tc.tile_pool(name="pf", bufs=1) as pf,
                tc.tile_pool(name="pf_out", bufs=2) as pf_out,
                tc.tile_pool(name="pf_tp", bufs=4, space="PSUM") as pf_tp,
                tc.tile_pool(name="pf_ctx", bufs=2, space="PSUM") as pf_ctx,
              ):
                cvT = pf.tile([128, NT, SQ], F32R)
                for t in range(NT):
                    nc.sync.dma_start(wv_sb[:, t, :], wv[128 * t:128 * (t + 1), :])
                for b in range(NB):
                    for t in range(NT):
                        tp = pf_tp.tile([128, 128], F32R, tag="tp")
                        nc.tensor.transpose(tp[:], cv_sb[b][:, 128 * t:128 * (t + 1)],
                                            ident_sb[:])
                        nc.vector.tensor_copy(cvT[:, t, 128 * b:128 * (b + 1)], tp[:])
                    for h_ in range(2):
                        hs = slice(512 * h_, 512 * (h_ + 1))
                        ps = pf_ctx.tile([128, 512], F32, tag="ctx")
                        for t in range(NT):
                            nc.tensor.matmul(ps[:], cvT[:, t, 128 * b:128 * (b + 1)],
                                             wv_sb[:, t, hs], start=(t == 0), stop=(t == NT - 1))
                        out_t = pf_out.tile([128, 512], F32, tag="out")
                        nc.vector.tensor_scalar_mul(out_t[:], ps[:], inv_sb[:, b:b + 1])
                        if apply_bv:
                            nc.vector.tensor_tensor(out_t[:], out_t[:], bv_bcast[:, hs],
                                                    op=mybir.AluOpType.add)
                        nc.sync.dma_start(out[128 * b:128 * (b + 1), hs], out_t[:])

    nc.compile()
    return nc


_CACHE = {}


def _get_program(apply_bq: bool, apply_bv: bool):
    key = (apply_bq, apply_bv)
    if key not in _CACHE:
        _CACHE[key] = build_program(apply_bq, apply_bv)
    return _CACHE[key]


def _prepare_in_maps(ins: dict) -> list:
    import ml_dtypes
    q = np.asarray(ins["q"], np.float32)
    k = np.asarray(ins["k"], np.float32)
    v = np.asarray(ins["v"], np.float32)
    assert q.shape == (S, D) and k.shape == (S, D) and v.shape == (S, D)

    qT = np.ascontiguousarray(q.T).astype(ml_dtypes.bfloat16)
    kT_bf = np.ascontiguousarray(k.T).astype(ml_dtypes.bfloat16)
    v_bf = v.astype(ml_dtypes.bfloat16)
    Wq = np.ascontiguousarray(np.asarray(ins["Wq"], np.float32)).astype(ml_dtypes.bfloat16)
    WkT = np.ascontiguousarray(np.asarray(ins["Wk"], np.float32).T).astype(ml_dtypes.bfloat16)
    Wv = np.ascontiguousarray(np.asarray(ins["Wv"], np.float32))
    bq = np.asarray(ins["bq"], np.float32).reshape(H)
    bv = np.asarray(ins["bv"], np.float32).reshape(H)

    bq_r = np.ascontiguousarray(bq.reshape(NJ, 128))
    bv_d = np.ascontiguousarray(bv.reshape(1, H))
    ident_np = np.eye(128, dtype=np.float32)

    in_maps = []
    for i in range(NCORES):
        sl = slice(SQ * i, SQ * (i + 1))
        in_maps.append({
            "qt": np.ascontiguousarray(qT[:, sl]),
            "ktf": kT_bf, "vf": v_bf,
            "wq": Wq, "wkt": WkT, "wv": Wv,
            "bq_r": bq_r, "bv_d": bv_d, "ident_d": ident_np,
            "ones_f": np.ones((1, 128), np.float32),
            "ones_bb": np.ones((128, 128), ml_dtypes.bfloat16),
        })
    return in_maps


def kernel(q, k, v, Wq, bq, Wk, bk, Wv, bv) -> np.ndarray:
    # bk contributes only per-row constants to scores and cancels in softmax.
    ins = {"q": q, "k": k, "v": v, "Wq": Wq, "bq": bq, "Wk": Wk,
           "Wv": Wv, "bv": bv}
    apply_bq = bool(np.any(np.asarray(bq)))
    apply_bv = bool(np.any(np.asarray(bv)))
    nc = _get_program(apply_bq, apply_bv)
    in_maps = _prepare_in_maps(ins)
    res = run_bass_kernel_spmd(nc, in_maps, core_ids=list(range(NCORES)))
    return np.concatenate([res.results[i]["out"] for i in range(NCORES)], axis=0)
